# revision 20
# baseline (speedup 1.0000x reference)
"""Averaged Hausdorff loss distributed Trainium2 kernel (8 NeuronCores).

reference:
    d[i,j] = ||set1_i - set2_j||  (sets are [8192, 128] f32)
    out = 0.5 * (sum_i min_j d + sum_j min_i d)

Strategy: shard set1 rows across the 8 cores (1024 rows each); every core
holds all of set2. Work with the NEGATED squared distance
    s[i,j] = 2*a_i.b_j - ||a_i||^2 - ||b_j||^2 = -d^2
so both reductions are maxes and sqrt applies only to the tiny results.
Per core (all compute bf16, fp32 psum accumulate):
  PE:   psum  = (2A)^T.T @ B^T   (K=128 main matmul)
        psum += ones^T  @ (-y2/128 replicated)  (K=128 bias matmul, folds
        -||b_j||^2 into psum at full matmul rate; a K=1 rank-1 matmul is
        ~4x slower per column on HW, so use a full-K ones matmul instead)
  ACT:  evict psum -> SBUF bf16 with per-partition bias -||a_i||^2.
  DVE:  col path: colacc = max(colacc, s_tile)   (elementwise, 2x mode)
        row path: log-fold s_tile 8192->1024 with tensor max, then
        reduce_max.
  GPSIMD: partition_all_reduce(max) over colacc -> per-j col max.
  Tail: negate+relu+sqrt, per-core row sqrt sum.
Host: elementwise min of the 8 col vectors + sum; sum of 8 row partials.
"""

import sys

sys.path.insert(0, "/opt/trn_rl_repo")

import ml_dtypes
import numpy as np

import concourse.bass as bass
import concourse.mybir as mybir
from concourse import bacc
from concourse.tile import TileContext

P = 128
N = 8192  # set1 rows (total)
M = 8192  # set2 rows
D = 128
NCORES = 8
NSH = N // NCORES  # 1024 rows per core
N_IT = NSH // P  # 8 i-tiles per core
JT = 512  # psum tile free width (one bank)
EV = 2048  # eviction group width (4 psum banks)
N_EV = M // EV  # 4 eviction groups per i-tile

BF = mybir.dt.bfloat16
F32 = mybir.dt.float32


def build_nc():
    nc = bacc.Bacc("TRN2")

    a2t = nc.declare_dram_parameter("a2t", [P, NSH], BF, isOutput=False)
    bt = nc.declare_dram_parameter("bt", [P, M], BF, isOutput=False)
    ny2r = nc.declare_dram_parameter("ny2r", [P, M], BF, isOutput=False)
    nx2 = nc.declare_dram_parameter("nx2", [P, N_IT], F32, isOutput=False)
    ident = nc.declare_dram_parameter("ident", [P, P], BF, isOutput=False)
    colout = nc.declare_dram_parameter("colout", [M], F32, isOutput=True)
    rowout = nc.declare_dram_parameter("rowout", [1], F32, isOutput=True)

    with TileContext(nc) as tc:
        with (
            tc.tile_pool(name="const", bufs=1) as cpool,
            tc.tile_pool(name="s", bufs=2) as spool,
            tc.tile_pool(name="fold", bufs=2) as fpool,
            tc.tile_pool(name="psum", bufs=2, space="PSUM") as ppool,
            tc.tile_pool(name="tail", bufs=1) as tpool,
        ):
            bt_sb = cpool.tile([P, M], BF, tag="bt")
            a2t_sb = cpool.tile([P, NSH], BF, tag="a2t")
            ny2r_sb = cpool.tile([P, M], BF, tag="ny2r")
            nx2_sb = cpool.tile([P, N_IT], F32, tag="nx2")
            ones_sb = cpool.tile([P, P], BF, tag="ones")
            ident_sb = cpool.tile([P, P], BF, tag="ident")
            colacc = cpool.tile([P, M], BF, tag="colacc")
            rowmax8 = cpool.tile([P, N_IT], F32, tag="rowmax8")

            # small inputs first, then big ones in fine-grained chunks spread
            # over the DMA rings (one dma_start = one ring at ~75 GB/s) so
            # the first j-groups land fast
            nc.vector.memset(ones_sb[:], 1.0)
            nc.sync.dma_start(out=a2t_sb[:], in_=a2t[:])
            nc.sync.dma_start(out=nx2_sb[:], in_=nx2[:])
            CH = 512
            for q in range(M // CH):
                nc.sync.dma_start(
                    out=bt_sb[:, q * CH : (q + 1) * CH],
                    in_=bt[:, q * CH : (q + 1) * CH],
                )
                nc.sync.dma_start(
                    out=ny2r_sb[:, q * CH : (q + 1) * CH],
                    in_=ny2r[:, q * CH : (q + 1) * CH],
                )
            nc.sync.dma_start(out=ident_sb[:], in_=ident[:])

            # PE prewarm: dummy matmuls on resident data while input DMAs run,
            # so the p-state ramp completes before the real work
            warm_sb = cpool.tile([P, JT], BF, tag="warm")
            nc.vector.memset(warm_sb[:], 0.0)
            warmps = ppool.tile([P, EV], F32, tag="pg")
            for w in range(24):
                nc.tensor.matmul(
                    warmps[:, (w % 4) * JT : (w % 4 + 1) * JT],
                    ones_sb[:],
                    warm_sb[:],
                    start=True,
                    stop=True,
                )

            last_reduce = None
            for it in range(N_IT):
                lhs = a2t_sb[:, it * P : (it + 1) * P]
                s_full = spool.tile([P, M], BF, tag="s")
                for g in range(N_EV):
                    pg = ppool.tile([P, EV], F32, tag="pg")
                    for jj in range(EV // JT):
                        jt = g * (EV // JT) + jj
                        nc.tensor.matmul(
                            pg[:, jj * JT : (jj + 1) * JT],
                            lhs,
                            bt_sb[:, jt * JT : (jt + 1) * JT],
                            start=True,
                            stop=False,
                        )
                    for jj in range(EV // JT):
                        jt = g * (EV // JT) + jj
                        nc.tensor.matmul(
                            pg[:, jj * JT : (jj + 1) * JT],
                            ones_sb[:],
                            ny2r_sb[:, jt * JT : (jt + 1) * JT],
                            start=False,
                            stop=True,
                        )
                    # evict 4 banks at once, adding -||a_i||^2 (per partition)
                    nc.scalar.activation(
                        s_full[:, g * EV : (g + 1) * EV],
                        pg[:],
                        mybir.ActivationFunctionType.Identity,
                        bias=nx2_sb[:, it : it + 1],
                        scale=1.0,
                    )

                # col path: running elementwise max over i-tiles
                if it == 0:
                    nc.vector.tensor_copy(colacc[:], s_full[:])
                else:
                    nc.vector.tensor_max(colacc[:], colacc[:], s_full[:])

                # row path: fold 8192 -> 1024 with TT max, then reduce
                f1 = fpool.tile([P, M // 2], BF, tag="f1")
                nc.vector.tensor_max(f1[:], s_full[:, 0 : M // 2], s_full[:, M // 2 : M])
                f2 = fpool.tile([P, M // 4], BF, tag="f2")
                nc.vector.tensor_max(f2[:], f1[:, 0 : M // 4], f1[:, M // 4 : M // 2])
                f3 = fpool.tile([P, M // 8], BF, tag="f3")
                nc.vector.tensor_max(f3[:], f2[:, 0 : M // 8], f2[:, M // 8 : M // 4])
                last_reduce = nc.vector.tensor_reduce(
                    rowmax8[:, it : it + 1],
                    f3[:],
                    axis=mybir.AxisListType.X,
                    op=mybir.AluOpType.max,
                )

            # ---- tail ----
            # col: partition max via PE transposes (2 waves of 32 tiles into
            # psum) + one strided DVE reduce per wave, then -x, relu, sqrt.
            # Emitted before the row tail: PE executes in program order and
            # the transposes only need colacc.
            colmaxT = tpool.tile([P, M // P], F32, tag="colmaxT")
            HW = 32  # transposes per wave
            for w in range(2):
                tps = ppool.tile([P, HW * P], BF, tag="pg")
                for t in range(HW):
                    tt = w * HW + t
                    nc.tensor.transpose(
                        tps[:, t * P : (t + 1) * P],
                        colacc[:, tt * P : (tt + 1) * P],
                        ident_sb[:],
                    )
                nc.vector.tensor_reduce(
                    colmaxT[:, w * HW : (w + 1) * HW],
                    tps[:].rearrange("p (t q) -> p t q", q=P),
                    axis=mybir.AxisListType.X,
                    op=mybir.AluOpType.max,
                )
            colsq = tpool.tile([P, M // P], F32, tag="colsq")
            nc.vector.tensor_scalar(
                colsq[:], colmaxT[:], -1.0, 0.0, mybir.AluOpType.mult, mybir.AluOpType.max
            )
            colsqrt = tpool.tile([P, M // P], F32, tag="colsqrt")
            nc.scalar.activation(colsqrt[:], colsq[:], mybir.ActivationFunctionType.Sqrt)
            # contiguous store; element (p, t) is column j = 128*t + p and the
            # host unpermutes (colout[64*p + t] = value for j)
            nc.sync.dma_start(
                out=colout.ap().rearrange("(p t) -> p t", p=P), in_=colsqrt[:]
            )

            # row: -x, relu, sqrt, sum over the core's 1024 rows; the
            # cross-partition sum is a K=128,N=1 matmul against ones.
            rowsq = tpool.tile([P, N_IT], F32, tag="rowsq")
            nc.vector.tensor_scalar(
                rowsq[:], rowmax8[:], -1.0, 0.0, mybir.AluOpType.mult, mybir.AluOpType.max
            )
            rowsqrt = tpool.tile([P, N_IT], F32, tag="rowsqrt")
            nc.scalar.activation(rowsqrt[:], rowsq[:], mybir.ActivationFunctionType.Sqrt)
            rowsum = tpool.tile([P, 1], F32, tag="rowsum")
            nc.vector.tensor_reduce(
                rowsum[:], rowsqrt[:], axis=mybir.AxisListType.X, op=mybir.AluOpType.add
            )
            ones1_f32 = tpool.tile([P, 1], F32, tag="ones1")
            nc.vector.memset(ones1_f32[:], 1.0)
            rowps = ppool.tile([1, 1], F32, tag="pg")
            nc.tensor.matmul(rowps[:], rowsum[:], ones1_f32[:], start=True, stop=True)
            rowtot = tpool.tile([1, 1], F32, tag="rowtot")
            nc.scalar.copy(rowtot[:], rowps[:])
            nc.sync.dma_start(
                out=rowout.ap().rearrange("(o p) -> o p", o=1), in_=rowtot[:]
            )

    nc.finalize()
    return nc


def make_in_maps(set1: np.ndarray, set2: np.ndarray):
    set1 = np.ascontiguousarray(set1, dtype=np.float32)
    set2 = np.ascontiguousarray(set2, dtype=np.float32)
    x2 = (set1.astype(np.float64) ** 2).sum(axis=1).astype(np.float32)  # [N]
    y2 = (set2.astype(np.float64) ** 2).sum(axis=1)  # [M] f64

    bt_bf = np.ascontiguousarray(set2.T).astype(ml_dtypes.bfloat16)  # [128, M]
    ny2r_bf = np.ascontiguousarray(
        np.broadcast_to((-y2 / P).astype(ml_dtypes.bfloat16), (P, M))
    )
    ident_bf = np.eye(P, dtype=ml_dtypes.bfloat16)

    in_maps = []
    for c in range(NCORES):
        rows = slice(c * NSH, (c + 1) * NSH)
        a2t_bf = np.ascontiguousarray((2.0 * set1[rows]).T).astype(ml_dtypes.bfloat16)
        nx2 = np.ascontiguousarray((-x2[rows]).reshape(N_IT, P).T)  # [p, t]
        in_maps.append(
            {"a2t": a2t_bf, "bt": bt_bf, "ny2r": ny2r_bf, "nx2": nx2, "ident": ident_bf}
        )
    return in_maps


def combine(results) -> np.float32:
    # colout is stored [p, t]-major; column j = 128*t + p lives at 64*p + t
    cols = np.stack(
        [np.asarray(r["colout"]).reshape(P, M // P).T.reshape(-1) for r in results]
    )  # [8, M]
    rows = np.array([np.asarray(r["rowout"]).reshape(()) for r in results])
    term2 = cols.min(axis=0).sum(dtype=np.float32)
    term1 = rows.sum(dtype=np.float32)
    return np.float32(0.5) * (np.float32(term1) + np.float32(term2))


_NC_CACHE = None


def _get_nc():
    global _NC_CACHE
    if _NC_CACHE is None:
        _NC_CACHE = build_nc()
    return _NC_CACHE


def run(set1, set2, trace=False, **trace_kwargs):
    from concourse.bass_utils import run_bass_kernel_spmd

    nc = _get_nc()
    in_maps = make_in_maps(set1, set2)
    res = run_bass_kernel_spmd(
        nc, in_maps, core_ids=list(range(NCORES)), trace=trace, **trace_kwargs
    )
    return combine(res.results), res


def kernel(set1: np.ndarray, set2: np.ndarray) -> np.ndarray:
    out, _ = run(set1, set2, trace=False)
    return np.asarray(out, dtype=np.float32)


# revision 21
# speedup vs baseline: 1.0935x; 1.0935x over previous
"""Averaged Hausdorff loss distributed Trainium2 kernel (8 NeuronCores).

reference:
    d[i,j] = ||set1_i - set2_j||  (sets are [8192, 128] f32)
    out = 0.5 * (sum_i min_j d + sum_j min_i d)

Strategy: shard set1 rows across the 8 cores (1024 rows each); every core
holds all of set2. Work with the NEGATED squared distance
    s[i,j] = 2*a_i.b_j - ||a_i||^2 - ||b_j||^2 = -d^2
so both reductions are maxes and sqrt applies only to the tiny results.
Per core (all compute bf16, fp32 psum accumulate):
  PE:   psum  = (2A)^T.T @ B^T   (K=128 main matmul)
        psum += ones^T  @ (-y2/128 replicated)  (K=128 bias matmul, folds
        -||b_j||^2 into psum at full matmul rate; a K=1 rank-1 matmul is
        ~4x slower per column on HW, so use a full-K ones matmul instead)
  ACT:  evict psum -> SBUF bf16 with per-partition bias -||a_i||^2.
  DVE:  col path: colacc = max(colacc, s_tile)   (elementwise, 2x mode)
        row path: log-fold s_tile 8192->1024 with tensor max, then
        reduce_max.
  GPSIMD: partition_all_reduce(max) over colacc -> per-j col max.
  Tail: negate+relu+sqrt, per-core row sqrt sum.
Host: elementwise min of the 8 col vectors + sum; sum of 8 row partials.
"""

import sys

sys.path.insert(0, "/opt/trn_rl_repo")

import ml_dtypes
import numpy as np

import concourse.bass as bass
import concourse.mybir as mybir
from concourse import bacc
from concourse.tile import TileContext

P = 128
N = 8192  # set1 rows (total)
M = 8192  # set2 rows
D = 128
NCORES = 8
NSH = N // NCORES  # 1024 rows per core
N_IT = NSH // P  # 8 i-tiles per core
JT = 512  # psum tile free width (one bank)
EV = 2048  # eviction group width (4 psum banks)
N_EV = M // EV  # 4 eviction groups per i-tile

BF = mybir.dt.bfloat16
F32 = mybir.dt.float32


def build_nc():
    nc = bacc.Bacc("TRN2")

    a2t = nc.declare_dram_parameter("a2t", [P, NSH], BF, isOutput=False)
    bt = nc.declare_dram_parameter("bt", [P, M], BF, isOutput=False)
    ny2r = nc.declare_dram_parameter("ny2r", [P, M], BF, isOutput=False)
    nx2 = nc.declare_dram_parameter("nx2", [P, N_IT], F32, isOutput=False)
    ident = nc.declare_dram_parameter("ident", [P, P], BF, isOutput=False)
    colout = nc.declare_dram_parameter("colout", [M], F32, isOutput=True)
    rowout = nc.declare_dram_parameter("rowout", [1], F32, isOutput=True)

    with TileContext(nc) as tc:
        with (
            tc.tile_pool(name="const", bufs=1) as cpool,
            tc.tile_pool(name="s", bufs=2) as spool,
            tc.tile_pool(name="fold", bufs=2) as fpool,
            tc.tile_pool(name="psum", bufs=2, space="PSUM") as ppool,
            tc.tile_pool(name="tail", bufs=1) as tpool,
        ):
            bt_sb = cpool.tile([P, M], BF, tag="bt")
            a2t_sb = cpool.tile([P, NSH], BF, tag="a2t")
            ny2r_sb = cpool.tile([P, M], BF, tag="ny2r")
            nx2_sb = cpool.tile([P, N_IT], F32, tag="nx2")
            ones_sb = cpool.tile([P, P], BF, tag="ones")
            ident_sb = cpool.tile([P, P], BF, tag="ident")
            colacc = cpool.tile([P, M], BF, tag="colacc")
            rowmax8 = cpool.tile([P, N_IT], F32, tag="rowmax8")

            # small inputs first, then big ones in fine-grained chunks spread
            # over the DMA rings (one dma_start = one ring at ~75 GB/s) so
            # the first j-groups land fast
            nc.vector.memset(ones_sb[:], 1.0)
            nc.sync.dma_start(out=a2t_sb[:], in_=a2t[:])
            nc.sync.dma_start(out=nx2_sb[:], in_=nx2[:])
            CH = 2048
            for q in range(M // CH):
                nc.sync.dma_start(
                    out=bt_sb[:, q * CH : (q + 1) * CH],
                    in_=bt[:, q * CH : (q + 1) * CH],
                )
                nc.sync.dma_start(
                    out=ny2r_sb[:, q * CH : (q + 1) * CH],
                    in_=ny2r[:, q * CH : (q + 1) * CH],
                )
            nc.sync.dma_start(out=ident_sb[:], in_=ident[:])

            # PE prewarm: dummy matmuls on resident data while input DMAs run,
            # so the p-state ramp completes before the real work
            warm_sb = cpool.tile([P, JT], BF, tag="warm")
            nc.vector.memset(warm_sb[:], 0.0)
            warmps = ppool.tile([P, EV], F32, tag="pg")
            for w in range(24):
                nc.tensor.matmul(
                    warmps[:, (w % 4) * JT : (w % 4 + 1) * JT],
                    ones_sb[:],
                    warm_sb[:],
                    start=True,
                    stop=True,
                )

            last_reduce = None
            for it in range(N_IT):
                lhs = a2t_sb[:, it * P : (it + 1) * P]
                s_full = spool.tile([P, M], BF, tag="s")
                for g in range(N_EV):
                    pg = ppool.tile([P, EV], F32, tag="pg")
                    for jj in range(EV // JT):
                        jt = g * (EV // JT) + jj
                        nc.tensor.matmul(
                            pg[:, jj * JT : (jj + 1) * JT],
                            lhs,
                            bt_sb[:, jt * JT : (jt + 1) * JT],
                            start=True,
                            stop=False,
                        )
                    for jj in range(EV // JT):
                        jt = g * (EV // JT) + jj
                        nc.tensor.matmul(
                            pg[:, jj * JT : (jj + 1) * JT],
                            ones_sb[:],
                            ny2r_sb[:, jt * JT : (jt + 1) * JT],
                            start=False,
                            stop=True,
                        )
                    # evict 4 banks at once, adding -||a_i||^2 (per partition)
                    nc.scalar.activation(
                        s_full[:, g * EV : (g + 1) * EV],
                        pg[:],
                        mybir.ActivationFunctionType.Identity,
                        bias=nx2_sb[:, it : it + 1],
                        scale=1.0,
                    )

                # col path: running elementwise max over i-tiles
                if it == 0:
                    nc.vector.tensor_copy(colacc[:], s_full[:])
                else:
                    nc.vector.tensor_max(colacc[:], colacc[:], s_full[:])

                # row path: fold 8192 -> 1024 with TT max, then reduce
                f1 = fpool.tile([P, M // 2], BF, tag="f1")
                nc.vector.tensor_max(f1[:], s_full[:, 0 : M // 2], s_full[:, M // 2 : M])
                f2 = fpool.tile([P, M // 4], BF, tag="f2")
                nc.vector.tensor_max(f2[:], f1[:, 0 : M // 4], f1[:, M // 4 : M // 2])
                f3 = fpool.tile([P, M // 8], BF, tag="f3")
                nc.vector.tensor_max(f3[:], f2[:, 0 : M // 8], f2[:, M // 8 : M // 4])
                last_reduce = nc.vector.tensor_reduce(
                    rowmax8[:, it : it + 1],
                    f3[:],
                    axis=mybir.AxisListType.X,
                    op=mybir.AluOpType.max,
                )

            # ---- tail ----
            # col: partition max via PE transposes (2 waves of 32 tiles into
            # psum) + one strided DVE reduce per wave, then -x, relu, sqrt.
            # Emitted before the row tail: PE executes in program order and
            # the transposes only need colacc.
            colmaxT = tpool.tile([P, M // P], F32, tag="colmaxT")
            HW = 32  # transposes per wave
            for w in range(2):
                tps = ppool.tile([P, HW * P], BF, tag="pg")
                for t in range(HW):
                    tt = w * HW + t
                    nc.tensor.transpose(
                        tps[:, t * P : (t + 1) * P],
                        colacc[:, tt * P : (tt + 1) * P],
                        ident_sb[:],
                    )
                nc.vector.tensor_reduce(
                    colmaxT[:, w * HW : (w + 1) * HW],
                    tps[:].rearrange("p (t q) -> p t q", q=P),
                    axis=mybir.AxisListType.X,
                    op=mybir.AluOpType.max,
                )
            colsq = tpool.tile([P, M // P], F32, tag="colsq")
            nc.vector.tensor_scalar(
                colsq[:], colmaxT[:], -1.0, 0.0, mybir.AluOpType.mult, mybir.AluOpType.max
            )
            colsqrt = tpool.tile([P, M // P], F32, tag="colsqrt")
            nc.scalar.activation(colsqrt[:], colsq[:], mybir.ActivationFunctionType.Sqrt)
            # contiguous store; element (p, t) is column j = 128*t + p and the
            # host unpermutes (colout[64*p + t] = value for j)
            nc.sync.dma_start(
                out=colout.ap().rearrange("(p t) -> p t", p=P), in_=colsqrt[:]
            )

            # row: -x, relu, sqrt, sum over the core's 1024 rows; the
            # cross-partition sum is a K=128,N=1 matmul against ones.
            rowsq = tpool.tile([P, N_IT], F32, tag="rowsq")
            nc.vector.tensor_scalar(
                rowsq[:], rowmax8[:], -1.0, 0.0, mybir.AluOpType.mult, mybir.AluOpType.max
            )
            rowsqrt = tpool.tile([P, N_IT], F32, tag="rowsqrt")
            nc.scalar.activation(rowsqrt[:], rowsq[:], mybir.ActivationFunctionType.Sqrt)
            rowsum = tpool.tile([P, 1], F32, tag="rowsum")
            nc.vector.tensor_reduce(
                rowsum[:], rowsqrt[:], axis=mybir.AxisListType.X, op=mybir.AluOpType.add
            )
            ones1_f32 = tpool.tile([P, 1], F32, tag="ones1")
            nc.vector.memset(ones1_f32[:], 1.0)
            rowps = ppool.tile([1, 1], F32, tag="pg")
            nc.tensor.matmul(rowps[:], rowsum[:], ones1_f32[:], start=True, stop=True)
            rowtot = tpool.tile([1, 1], F32, tag="rowtot")
            nc.scalar.copy(rowtot[:], rowps[:])
            nc.sync.dma_start(
                out=rowout.ap().rearrange("(o p) -> o p", o=1), in_=rowtot[:]
            )

    nc.finalize()
    return nc


def make_in_maps(set1: np.ndarray, set2: np.ndarray):
    set1 = np.ascontiguousarray(set1, dtype=np.float32)
    set2 = np.ascontiguousarray(set2, dtype=np.float32)
    x2 = (set1.astype(np.float64) ** 2).sum(axis=1).astype(np.float32)  # [N]
    y2 = (set2.astype(np.float64) ** 2).sum(axis=1)  # [M] f64

    bt_bf = np.ascontiguousarray(set2.T).astype(ml_dtypes.bfloat16)  # [128, M]
    ny2r_bf = np.ascontiguousarray(
        np.broadcast_to((-y2 / P).astype(ml_dtypes.bfloat16), (P, M))
    )
    ident_bf = np.eye(P, dtype=ml_dtypes.bfloat16)

    in_maps = []
    for c in range(NCORES):
        rows = slice(c * NSH, (c + 1) * NSH)
        a2t_bf = np.ascontiguousarray((2.0 * set1[rows]).T).astype(ml_dtypes.bfloat16)
        nx2 = np.ascontiguousarray((-x2[rows]).reshape(N_IT, P).T)  # [p, t]
        in_maps.append(
            {"a2t": a2t_bf, "bt": bt_bf, "ny2r": ny2r_bf, "nx2": nx2, "ident": ident_bf}
        )
    return in_maps


def combine(results) -> np.float32:
    # colout is stored [p, t]-major; column j = 128*t + p lives at 64*p + t
    cols = np.stack(
        [np.asarray(r["colout"]).reshape(P, M // P).T.reshape(-1) for r in results]
    )  # [8, M]
    rows = np.array([np.asarray(r["rowout"]).reshape(()) for r in results])
    term2 = cols.min(axis=0).sum(dtype=np.float32)
    term1 = rows.sum(dtype=np.float32)
    return np.float32(0.5) * (np.float32(term1) + np.float32(term2))


_NC_CACHE = None


def _get_nc():
    global _NC_CACHE
    if _NC_CACHE is None:
        _NC_CACHE = build_nc()
    return _NC_CACHE


def run(set1, set2, trace=False, **trace_kwargs):
    from concourse.bass_utils import run_bass_kernel_spmd

    nc = _get_nc()
    in_maps = make_in_maps(set1, set2)
    res = run_bass_kernel_spmd(
        nc, in_maps, core_ids=list(range(NCORES)), trace=trace, **trace_kwargs
    )
    return combine(res.results), res


def kernel(set1: np.ndarray, set2: np.ndarray) -> np.ndarray:
    out, _ = run(set1, set2, trace=False)
    return np.asarray(out, dtype=np.float32)
